# revision 21
# baseline (speedup 1.0000x reference)
"""Trainium2 Bass kernel for GroupAttention.

Reference computation (B=4, N=8192, C=1024, H=16 heads, Dh=64, groups of
g=4 consecutive tokens):
    qkv = x @ w_qkv                      # [B,N,3C]
    per (batch, group, head): S = (q*Dh^-0.5) @ k.T   (4x4)
    P = softmax(S, axis=-1); o = P @ v
    y = o @ w_proj + b_proj

Strategy: data-parallel over the 32768 tokens -> 4096 tokens/core on 8
NeuronCores (group locality preserved).

x is pre-transposed on the host to feature-major [C, tokens] with the
group-major token order baked in (col = w*512 + n*128 + gi for token
w*512 + gi*4 + n), so the qkv matmul consumes it directly as lhsT with
no on-chip transposes. Per core, per 512-token window (= 128 groups):
  - DMA xT window (bf16, host-cast): 8 contraction tiles [128c, 512t].
  - qkv matmul -> PSUM [128 groups, outc]; ACT copies+casts to bf16
    SBUF tiles Q/K/V laid out [group, (pos, head, dh)]. Double-buffered
    so window w+1 qkv (PE) overlaps window w attention (DVE/Pool).
  - Attention split across VectorE and GpSimdE: score chains (elementwise
    mult + a pairwise bf16 add-tree over dh; TensorReduce has no 2x mode,
    the tree does) for key positions {0,1} on DVE and {2,3} on GpSimd
    (m3's tree back on DVE); exp on ACT; z/recip/normalize on DVE; AV as
    broadcast mult + add-trees split by head half (heads 0-7 DVE, 8-15
    GpSimd), each engine writing its half of O directly.
    The k/v qkv chunks are ordered per key position so score chain m
    starts well before its window's qkv matmuls finish.
  - PE-transpose O back to feature-major, proj matmul; bias added
    during the PSUM->SBUF copy as a DVE tensor_add against a
    DMA-replicated [128, C] bias tile; DMA out (fp32).

The 1/sqrt(Dh) scale is folded into the q-columns of w_qkv on the host.
Matmul/attention inputs are bf16 (cast host-side); accumulations are
fp32 (PSUM / DVE internal), except the score add-tree which rounds to
bf16 per level (~0.8% extra relative error on exp(S), well within the
2e-2 gate).
"""

import numpy as np
import ml_dtypes

import concourse.bass as bass
import concourse.bacc as bacc
import concourse.mybir as mybir
import concourse.tile as tile
from concourse.bass_utils import run_bass_kernel_spmd

BF16 = mybir.dt.bfloat16
F32 = mybir.dt.float32
AF = mybir.ActivationFunctionType
ALU = mybir.AluOpType
AX = mybir.AxisListType

B, N, C = 4, 8192, 1024
H, DH, GSZ = 16, 64, 4
NCORES = 8
T_CORE = (B * N) // NCORES  # 4096 tokens per core
WIN = 512                   # tokens per window (= 128 groups)
G128 = WIN // GSZ           # 128 groups per window
KT = C // 128               # 8 contraction tiles of 128
OUT3 = 3 * C                # 3072
NCH = OUT3 // 512           # 6 qkv output chunks of 512


def _score_chain(eng, m, scratch, qb, kb, s_f, tree_eng=None, exp_out=None, nc=None):
    """scores for key position m on engine `eng` (nc.vector / nc.gpsimd):
    prod = Q * K[m] broadcast over n, then a pairwise add-tree over dh.
    Uses the 8KB mul tile A plus a 4KB tree tile B; levels 2+ ping-pong
    through disjoint regions of A once the mul output is consumed."""
    a_t = scratch.tile([128, 4 * C], BF16, tag="scrA", bufs=2)
    b_t = scratch.tile([128, 2048], BF16, tag="scrB", bufs=1)
    q_v = qb[:].rearrange("p (n h d) -> p n h d", n=GSZ, h=H)
    k_v = (
        kb[:, m * C:(m + 1) * C]
        .rearrange("p (h d) -> p h d", h=H)
        .unsqueeze(1)
        .broadcast_to([128, GSZ, H, DH])
    )
    a_v = a_t[:].rearrange("p (n h d) -> p n h d", n=GSZ, h=H)
    eng.tensor_mul(a_v, q_v, k_v)
    if tree_eng is not None:
        eng = tree_eng  # run the add-tree on a different engine
    # tree levels: seg widths 32,16,8,4,2 -> final f32 add into s_f
    segv = lambda t, off, w: t[:, off: off + 64 * 2 * w].rearrange(
        "p (s d) -> p s d", d=2 * w
    )
    outv = lambda t, off, w: t[:, off: off + 64 * w].rearrange(
        "p (s d) -> p s d", d=w
    )
    # L1: A[0:4096] -> B[0:2048]
    src = segv(a_t, 0, 32)
    eng.tensor_add(outv(b_t, 0, 32), src[:, :, 0:32], src[:, :, 32:64])
    # L2: B[0:2048] -> A[0:1024]   (A's mul output has been consumed)
    src = segv(b_t, 0, 16)
    eng.tensor_add(outv(a_t, 0, 16), src[:, :, 0:16], src[:, :, 16:32])
    # L3: A[0:1024] -> A[2048:2560]
    src = segv(a_t, 0, 8)
    eng.tensor_add(outv(a_t, 2048, 8), src[:, :, 0:8], src[:, :, 8:16])
    # L4: A[2048:2560] -> A[0:256]
    src = segv(a_t, 2048, 4)
    eng.tensor_add(outv(a_t, 0, 4), src[:, :, 0:4], src[:, :, 4:8])
    # L5: A[0:256] -> A[2048:2176]
    src = segv(a_t, 0, 2)
    eng.tensor_add(outv(a_t, 2048, 2), src[:, :, 0:2], src[:, :, 2:4])
    # L6: A[2048:2176] strided pair -> s_f (f32)
    eng.tensor_add(
        s_f[:, m * 64:(m + 1) * 64],
        a_t[:, 2048:2176:2],
        a_t[:, 2049:2176:2],
    )
    if exp_out is not None:
        nc.scalar.activation(
            exp_out[:, m * 64:(m + 1) * 64], s_f[:, m * 64:(m + 1) * 64],
            AF.Exp,
        )


def group_attn_kernel(tc, y, xt, wqkv, wproj, bias, ident, t_core=T_CORE):
    """Emit the per-core kernel. All args are DRAM APs:
    y [t_core, C] f32 out; xt [C, t_core] bf16 (feature-major, group-major
    token order); wqkv [C, 3C] bf16 (q cols pre-scaled); wproj [C, C]
    bf16; bias [1, C] bf16; ident [128,128] bf16.
    """
    nc = tc.nc
    nwin = t_core // WIN

    from contextlib import ExitStack

    with ExitStack() as ctx:
        ep = ctx.enter_context

        const = ep(tc.tile_pool(name="const", bufs=1))
        xpool = ep(tc.tile_pool(name="x", bufs=2))
        qpool = ep(tc.tile_pool(name="qb", bufs=2))
        kpool = ep(tc.tile_pool(name="kb", bufs=2))
        vpool = ep(tc.tile_pool(name="vb", bufs=2))
        spool = ep(tc.tile_pool(name="soft", bufs=2))
        dscr = ep(tc.tile_pool(name="dscr", bufs=1))   # DVE scratch
        pscr = ep(tc.tile_pool(name="pscr", bufs=1))   # Pool scratch
        opool = ep(tc.tile_pool(name="o", bufs=1))
        otpool = ep(tc.tile_pool(name="ot", bufs=2))
        ypool = ep(tc.tile_pool(name="y", bufs=2))

        ps_qkv = ep(tc.tile_pool(name="ps_qkv", bufs=3, space="PSUM"))
        ps_t = ep(tc.tile_pool(name="ps_t", bufs=2, space="PSUM"))
        ps_y = ep(tc.tile_pool(name="ps_y", bufs=2, space="PSUM"))

        # ---- startup DMAs, ordered so the first qkv matmul is gated only
        # by xw(0) + wqkv chunk 0 (~6us), not the full weight load ----
        def load_xw(w):
            # issued on the ACT HWDGE queue so it runs parallel to the
            # weight DMAs on the SP queue
            t = xpool.tile([128, KT * WIN], BF16, tag="xw")
            nc.scalar.dma_start(
                t[:].rearrange("p (k t) -> p k t", k=KT),
                xt.rearrange("(k p) t -> p k t", p=128)[
                    :, :, w * WIN:(w + 1) * WIN
                ],
            )
            return t

        # first window's x goes via the GpSimd SWDGE queue (idle at start)
        # so it overlaps wqkv chunk 0 on SP and LoadActFuncSet on ACT.
        # Both first-window loads are split in half so the first matmul
        # (k=0) is gated by ~2us of DMA, not the full tiles.
        xw0 = xpool.tile([128, KT * WIN], BF16, tag="xw")
        for half in range(2):
            nc.gpsimd.dma_start(
                xw0[:, half * 4 * WIN:(half + 1) * 4 * WIN].rearrange(
                    "p (k t) -> p k t", k=KT // 2
                ),
                xt.rearrange("(k p) t -> p k t", p=128)[
                    :, half * 4:(half + 1) * 4, 0:WIN
                ],
            )
        wqkv_ch = []
        for ch in range(NCH):
            t = const.tile([128, KT * 512], BF16, tag=f"wqkv{ch}")
            nhalf = 2 if ch == 0 else 1
            for half in range(nhalf):
                kspan = KT // nhalf
                nc.sync.dma_start(
                    t[:, half * kspan * 512:(half + 1) * kspan * 512].rearrange(
                        "p (k c) -> p k c", k=kspan
                    ),
                    wqkv.rearrange("(k p) c -> p k c", p=128)[
                        :, half * kspan:(half + 1) * kspan,
                        ch * 512:(ch + 1) * 512
                    ],
                )
            wqkv_ch.append(t)
        wproj_sb = const.tile([128, KT * C], BF16)     # 16KB/part
        nc.sync.dma_start(
            wproj_sb[:].rearrange("p (k c) -> p k c", k=KT),
            wproj.rearrange("(k p) c -> p k c", p=128),
        )
        bias128 = const.tile([128, C], BF16)
        nc.sync.dma_start(bias128[:], bias.broadcast_to([128, C]))
        ident_sb = const.tile([128, 128], BF16)
        nc.sync.dma_start(ident_sb[:], ident[:])

        for w in range(nwin):
            # ---- load xT window: KT tiles [128c, 512 tok] (group-major) ----
            xw = xw0 if w == 0 else load_xw(w)

            # ---- qkv matmuls, group-major output ----
            qb = qpool.tile([128, 4 * C], BF16)   # [g, (n, h, dh)]
            kb = kpool.tile([128, 4 * C], BF16)   # [g, (m, h, dh)]
            vb = vpool.tile([128, 4 * C], BF16)   # [g, (m, h, dh)]
            dest_of = {0: qb, 1: kb, 2: vb}
            # chunk order: all q (attention needs full Q), then k per key
            # position m, then v per m -- so score chain m can start well
            # before the window's qkv finishes.
            def qkv_chunks(order):
                for ch, n in order:
                    ps = ps_qkv.tile([128, 512], F32)
                    for k in range(KT):
                        nc.tensor.matmul(
                            ps[:],
                            lhsT=xw[:, k * WIN + n * 128:(k * WIN) + (n + 1) * 128],
                            rhs=wqkv_ch[ch][:, k * 512:(k + 1) * 512],
                            start=(k == 0),
                            stop=(k == KT - 1),
                        )
                    which, hblk = divmod(ch, 2)
                    dst = dest_of[which][:, n * C + hblk * 512: n * C + (hblk + 1) * 512]
                    nc.scalar.copy(dst, ps[:])

            qkv_chunks([(ch, n) for ch in (0, 1) for n in range(GSZ)])
            qkv_chunks([(ch, n) for n in (2, 0, 1, 3) for ch in (2, 3)])

            # ---- attention (per window, all 16 heads) ----
            # scores: S[g, (m, n, h)] = sum_dh Q[g,n,h,:] * K[g,m,h,:]
            # m chains 0,1 on DVE; 2,3 on GpSimd
            s_f = spool.tile([128, 256], F32, tag="s")
            e_f = spool.tile([128, 256], F32, tag="e")
            # exp is emitted per-m right after each chain (softmax over m,
            # no max-subtraction: |S| is O(5) here); m1's add-tree runs on
            # GpSimd, which idles while DVE is the serial path
            _score_chain(nc.vector, 0, dscr, qb, kb, s_f, exp_out=e_f, nc=nc)
            _score_chain(nc.vector, 1, dscr, qb, kb, s_f, tree_eng=nc.gpsimd,
                         exp_out=e_f, nc=nc)
            _score_chain(nc.gpsimd, 2, pscr, qb, kb, s_f, exp_out=e_f, nc=nc)
            _score_chain(nc.vector, 3, dscr, qb, kb, s_f, exp_out=e_f, nc=nc)

            e_nhm = e_f[:].rearrange("p (m n h) -> p n h m", m=GSZ, n=GSZ)
            z_f = spool.tile([128, 64], F32, tag="z")
            nc.vector.tensor_reduce(
                z_f[:].rearrange("p (n h) -> p n h", n=GSZ), e_nhm,
                axis=AX.X, op=ALU.add,
            )
            r_f = spool.tile([128, 64], F32, tag="r")
            nc.vector.reciprocal(r_f[:], z_f[:])
            pb = spool.tile([128, 256], BF16, tag="pb")  # [g, (n, h, m)]
            r_v = (
                r_f[:].rearrange("p (n h) -> p n h", n=GSZ)
                .unsqueeze(3)
                .broadcast_to([128, GSZ, H, GSZ])
            )
            pb_v = pb[:].rearrange("p (n h m) -> p n h m", n=GSZ, h=H)
            nc.vector.tensor_mul(pb_v, e_nhm, r_v)

            # v chunks emitted after the softmax ops so exp doesn't queue
            # behind the v copies on ACT
            qkv_chunks([(ch, n) for n in range(GSZ) for ch in (4, 5)])

            # AV: O[g, (n, h, d)] = sum_m P[g,n,h,m] * V[g,m,h,:]
            # Split by head-half: heads 0..7 on DVE, 8..15 on GpSimd.
            # Each engine computes its half of all 4 m-products and the
            # add tree, writing its half of ob directly (no combine op).
            ob = opool.tile([128, 4 * C], BF16)
            HH = H // 2

            def pv_mul_half(eng, m, dst_v, h0):
                v_v = (
                    vb[:, m * C + h0 * DH: m * C + (h0 + HH) * DH]
                    .rearrange("p (h d) -> p h d", h=HH)
                    .unsqueeze(1)
                    .broadcast_to([128, GSZ, HH, DH])
                )
                p_v = (
                    pb[:].rearrange("p (n h m) -> p n h m", n=GSZ, h=H)[
                        :, :, h0:h0 + HH, m
                    ]
                    .unsqueeze(3)
                    .broadcast_to([128, GSZ, HH, DH])
                )
                eng.tensor_mul(dst_v, v_v, p_v)

            def av_half(eng, scratch, h0):
                half = lambda t, i: t[:, i * 2048:(i + 1) * 2048].rearrange(
                    "p (n h d) -> p n h d", n=GSZ, h=HH
                )
                t1 = scratch.tile([128, 4 * C], BF16, tag="scrA", bufs=2)
                t2 = scratch.tile([128, 4 * C], BF16, tag="scrA", bufs=2)
                u = scratch.tile([128, 2048], BF16, tag="scrB", bufs=1)
                pv_mul_half(eng, 0, half(t1, 0), h0)
                pv_mul_half(eng, 1, half(t1, 1), h0)
                pv_mul_half(eng, 2, half(t2, 0), h0)
                pv_mul_half(eng, 3, half(t2, 1), h0)
                u_v = u[:].rearrange("p (n h d) -> p n h d", n=GSZ, h=HH)
                eng.tensor_add(u_v, half(t1, 0), half(t1, 1))
                eng.tensor_add(half(t1, 0), half(t2, 0), half(t2, 1))
                ob_v = ob[:].rearrange("p (n h d) -> p n h d", n=GSZ, h=H)[
                    :, :, h0:h0 + HH, :
                ]
                eng.tensor_add(ob_v, u_v, half(t1, 0))

            av_half(nc.vector, dscr, 0)
            av_half(nc.gpsimd, pscr, HH)

            # ---- transpose O to feature-major oT: KT tiles [128c, (n, g)] ----
            ot = []
            for j in range(KT):
                pst = ps_t.tile([128, WIN], BF16)
                for n in range(GSZ):
                    nc.tensor.transpose(
                        pst[:, n * 128:(n + 1) * 128],
                        ob[:, n * C + j * 128: n * C + (j + 1) * 128],
                        ident_sb[:],
                    )
                otj = otpool.tile([128, WIN], BF16, tag=f"ot{j}")
                nc.scalar.copy(otj[:], pst[:])
                ot.append(otj)

            # ---- proj matmul; bias added in the PSUM->SBUF copy; DMA out ----
            for n in range(GSZ):
                for ch in range(2):
                    psy = ps_y.tile([128, 512], F32)
                    for k in range(KT):
                        nc.tensor.matmul(
                            psy[:],
                            lhsT=ot[k][:, n * 128:(n + 1) * 128],
                            rhs=wproj_sb[:, k * C + ch * 512: k * C + (ch + 1) * 512],
                            start=(k == 0),
                            stop=(k == KT - 1),
                        )
                    y_t = ypool.tile([128, 512], F32)
                    nc.vector.tensor_add(
                        y_t[:], psy[:], bias128[:, ch * 512:(ch + 1) * 512]
                    )
                    nc.sync.dma_start(
                        y[w * WIN + n: w * WIN + WIN: GSZ,
                          ch * 512:(ch + 1) * 512],
                        y_t[:],
                    )


def build_nc(t_core=T_CORE):
    nc = bacc.Bacc("TRN2", target_bir_lowering=False, debug=False)
    xt_d = nc.dram_tensor("xt", [C, t_core], BF16, kind="ExternalInput")
    wqkv_d = nc.dram_tensor("wqkv", [C, OUT3], BF16, kind="ExternalInput")
    wproj_d = nc.dram_tensor("wproj", [C, C], BF16, kind="ExternalInput")
    bias_d = nc.dram_tensor("bias", [1, C], BF16, kind="ExternalInput")
    ident_d = nc.dram_tensor("ident", [128, 128], BF16, kind="ExternalInput")
    y_d = nc.dram_tensor("y", [t_core, C], F32, kind="ExternalOutput")
    with tile.TileContext(nc) as tc:
        group_attn_kernel(
            tc, y_d[:], xt_d[:], wqkv_d[:], wproj_d[:], bias_d[:],
            ident_d[:], t_core=t_core,
        )
    nc.compile()
    return nc


def make_in_maps(x, w_qkv, w_proj, b_proj):
    bf = ml_dtypes.bfloat16
    xf = np.ascontiguousarray(np.asarray(x, dtype=np.float32)).reshape(-1, C)
    wq = np.array(w_qkv, dtype=np.float32, copy=True)
    wq[:, :C] *= DH ** -0.5  # fold attention scale into q columns
    wqb = wq.astype(bf)
    wpb = np.asarray(w_proj, dtype=np.float32).astype(bf)
    bb = np.asarray(b_proj, dtype=np.float32).reshape(1, C).astype(bf)
    ident = np.eye(128, dtype=np.float32).astype(bf)
    xb = xf.astype(bf)
    in_maps = []
    for i in range(NCORES):
        xc = xb[i * T_CORE:(i + 1) * T_CORE]          # [4096, C]
        # feature-major with group-major token order: col = w*512+n*128+gi
        # for token w*512+gi*4+n
        xg = xc.reshape(T_CORE // WIN, G128, GSZ, C)  # [w, gi, n, c]
        xg = np.ascontiguousarray(np.transpose(xg, (3, 0, 2, 1)))
        in_maps.append({
            "xt": xg.reshape(C, T_CORE),
            "wqkv": wqb,
            "wproj": wpb,
            "bias": bb,
            "ident": ident,
        })
    return in_maps


_NC_CACHE = {}


def _get_nc():
    if "nc" not in _NC_CACHE:
        _NC_CACHE["nc"] = build_nc()
    return _NC_CACHE["nc"]


def kernel(x, w_qkv, w_proj, b_proj, causal=0, **_unused):
    nc = _get_nc()
    in_maps = make_in_maps(x, w_qkv, w_proj, b_proj)
    res = run_bass_kernel_spmd(nc, in_maps, core_ids=list(range(NCORES)))
    y = np.concatenate([r["y"] for r in res.results], axis=0)
    return y.reshape(B, N, C).astype(np.float32)


# revision 22
# speedup vs baseline: 1.0245x; 1.0245x over previous
"""Trainium2 Bass kernel for GroupAttention.

Reference computation (B=4, N=8192, C=1024, H=16 heads, Dh=64, groups of
g=4 consecutive tokens):
    qkv = x @ w_qkv                      # [B,N,3C]
    per (batch, group, head): S = (q*Dh^-0.5) @ k.T   (4x4)
    P = softmax(S, axis=-1); o = P @ v
    y = o @ w_proj + b_proj

Strategy: data-parallel over the 32768 tokens -> 4096 tokens/core on 8
NeuronCores (group locality preserved).

x is pre-transposed on the host to feature-major [C, tokens] with the
group-major token order baked in (col = w*512 + n*128 + gi for token
w*512 + gi*4 + n), so the qkv matmul consumes it directly as lhsT with
no on-chip transposes. Per core, per 512-token window (= 128 groups):
  - DMA xT window (bf16, host-cast): 8 contraction tiles [128c, 512t].
  - qkv matmul -> PSUM [128 groups, outc]; ACT copies+casts to bf16
    SBUF tiles Q/K/V laid out [group, (pos, head, dh)]. Double-buffered
    so window w+1 qkv (PE) overlaps window w attention (DVE/Pool).
  - Attention split across VectorE and GpSimdE: score chains (elementwise
    mult + a pairwise bf16 add-tree over dh; TensorReduce has no 2x mode,
    the tree does) for key positions {0,1} on DVE and {2,3} on GpSimd
    (m3's tree back on DVE); exp on ACT; z/recip/normalize on DVE; AV as
    broadcast mult + add-trees split by head half (heads 0-7 DVE, 8-15
    GpSimd), each engine writing its half of O directly.
    The k/v qkv chunks are ordered per key position so score chain m
    starts well before its window's qkv matmuls finish.
  - DMA-xbar-transpose O back to feature-major, proj matmul; bias added
    during the PSUM->SBUF copy as a DVE tensor_add against a
    DMA-replicated [128, C] bias tile; DMA out (fp32).

The 1/sqrt(Dh) scale is folded into the q-columns of w_qkv on the host.
Matmul/attention inputs are bf16 (cast host-side); accumulations are
fp32 (PSUM / DVE internal), except the score add-tree which rounds to
bf16 per level (~0.8% extra relative error on exp(S), well within the
2e-2 gate).
"""

import numpy as np
import ml_dtypes

import concourse.bass as bass
import concourse.bacc as bacc
import concourse.mybir as mybir
import concourse.tile as tile
from concourse.bass_utils import run_bass_kernel_spmd

BF16 = mybir.dt.bfloat16
F32 = mybir.dt.float32
AF = mybir.ActivationFunctionType
ALU = mybir.AluOpType
AX = mybir.AxisListType

B, N, C = 4, 8192, 1024
H, DH, GSZ = 16, 64, 4
NCORES = 8
T_CORE = (B * N) // NCORES  # 4096 tokens per core
WIN = 512                   # tokens per window (= 128 groups)
G128 = WIN // GSZ           # 128 groups per window
KT = C // 128               # 8 contraction tiles of 128
OUT3 = 3 * C                # 3072
NCH = OUT3 // 512           # 6 qkv output chunks of 512


def _score_chain(eng, m, scratch, qb, kb, s_f, tree_eng=None, exp_out=None, nc=None):
    """scores for key position m on engine `eng` (nc.vector / nc.gpsimd):
    prod = Q * K[m] broadcast over n, then a pairwise add-tree over dh.
    Uses the 8KB mul tile A plus a 4KB tree tile B; levels 2+ ping-pong
    through disjoint regions of A once the mul output is consumed."""
    a_t = scratch.tile([128, 4 * C], BF16, tag="scrA", bufs=2)
    b_t = scratch.tile([128, 2048], BF16, tag="scrB", bufs=1)
    q_v = qb[:].rearrange("p (n h d) -> p n h d", n=GSZ, h=H)
    k_v = (
        kb[:, m * C:(m + 1) * C]
        .rearrange("p (h d) -> p h d", h=H)
        .unsqueeze(1)
        .broadcast_to([128, GSZ, H, DH])
    )
    a_v = a_t[:].rearrange("p (n h d) -> p n h d", n=GSZ, h=H)
    eng.tensor_mul(a_v, q_v, k_v)
    if tree_eng is not None:
        eng = tree_eng  # run the add-tree on a different engine
    # tree levels: seg widths 32,16,8,4,2 -> final f32 add into s_f
    segv = lambda t, off, w: t[:, off: off + 64 * 2 * w].rearrange(
        "p (s d) -> p s d", d=2 * w
    )
    outv = lambda t, off, w: t[:, off: off + 64 * w].rearrange(
        "p (s d) -> p s d", d=w
    )
    # L1: A[0:4096] -> B[0:2048]
    src = segv(a_t, 0, 32)
    eng.tensor_add(outv(b_t, 0, 32), src[:, :, 0:32], src[:, :, 32:64])
    # L2: B[0:2048] -> A[0:1024]   (A's mul output has been consumed)
    src = segv(b_t, 0, 16)
    eng.tensor_add(outv(a_t, 0, 16), src[:, :, 0:16], src[:, :, 16:32])
    # L3: A[0:1024] -> A[2048:2560]
    src = segv(a_t, 0, 8)
    eng.tensor_add(outv(a_t, 2048, 8), src[:, :, 0:8], src[:, :, 8:16])
    # L4: A[2048:2560] -> A[0:256]
    src = segv(a_t, 2048, 4)
    eng.tensor_add(outv(a_t, 0, 4), src[:, :, 0:4], src[:, :, 4:8])
    # L5: A[0:256] -> A[2048:2176]
    src = segv(a_t, 0, 2)
    eng.tensor_add(outv(a_t, 2048, 2), src[:, :, 0:2], src[:, :, 2:4])
    # L6: A[2048:2176] strided pair -> s_f (f32)
    eng.tensor_add(
        s_f[:, m * 64:(m + 1) * 64],
        a_t[:, 2048:2176:2],
        a_t[:, 2049:2176:2],
    )
    if exp_out is not None:
        nc.scalar.activation(
            exp_out[:, m * 64:(m + 1) * 64], s_f[:, m * 64:(m + 1) * 64],
            AF.Exp,
        )


def group_attn_kernel(tc, y, xt, wqkv, wproj, bias, t_core=T_CORE):
    """Emit the per-core kernel. All args are DRAM APs:
    y [t_core, C] f32 out; xt [C, t_core] bf16 (feature-major, group-major
    token order); wqkv [C, 3C] bf16 (q cols pre-scaled); wproj [C, C]
    bf16; bias [1, C] bf16.
    """
    nc = tc.nc
    nwin = t_core // WIN

    from contextlib import ExitStack

    with ExitStack() as ctx:
        ep = ctx.enter_context

        const = ep(tc.tile_pool(name="const", bufs=1))
        xpool = ep(tc.tile_pool(name="x", bufs=2))
        qpool = ep(tc.tile_pool(name="qb", bufs=2))
        kpool = ep(tc.tile_pool(name="kb", bufs=2))
        vpool = ep(tc.tile_pool(name="vb", bufs=2))
        spool = ep(tc.tile_pool(name="soft", bufs=2))
        dscr = ep(tc.tile_pool(name="dscr", bufs=1))   # DVE scratch
        pscr = ep(tc.tile_pool(name="pscr", bufs=1))   # Pool scratch
        opool = ep(tc.tile_pool(name="o", bufs=1))
        otpool = ep(tc.tile_pool(name="ot", bufs=2))
        ypool = ep(tc.tile_pool(name="y", bufs=2))

        ps_qkv = ep(tc.tile_pool(name="ps_qkv", bufs=3, space="PSUM"))
        ps_y = ep(tc.tile_pool(name="ps_y", bufs=2, space="PSUM"))

        # ---- startup DMAs, ordered so the first qkv matmul is gated only
        # by xw(0) + wqkv chunk 0 (~6us), not the full weight load ----
        def load_xw(w):
            # issued on the ACT HWDGE queue so it runs parallel to the
            # weight DMAs on the SP queue
            t = xpool.tile([128, KT * WIN], BF16, tag="xw")
            nc.scalar.dma_start(
                t[:].rearrange("p (k t) -> p k t", k=KT),
                xt.rearrange("(k p) t -> p k t", p=128)[
                    :, :, w * WIN:(w + 1) * WIN
                ],
            )
            return t

        # first window's x goes via the GpSimd SWDGE queue (idle at start)
        # so it overlaps wqkv chunk 0 on SP and LoadActFuncSet on ACT.
        # Both first-window loads are split in half so the first matmul
        # (k=0) is gated by ~2us of DMA, not the full tiles.
        xw0 = xpool.tile([128, KT * WIN], BF16, tag="xw")
        for half in range(2):
            nc.gpsimd.dma_start(
                xw0[:, half * 4 * WIN:(half + 1) * 4 * WIN].rearrange(
                    "p (k t) -> p k t", k=KT // 2
                ),
                xt.rearrange("(k p) t -> p k t", p=128)[
                    :, half * 4:(half + 1) * 4, 0:WIN
                ],
            )
        wqkv_ch = []
        for ch in range(NCH):
            t = const.tile([128, KT * 512], BF16, tag=f"wqkv{ch}")
            nhalf = 2 if ch == 0 else 1
            for half in range(nhalf):
                kspan = KT // nhalf
                nc.sync.dma_start(
                    t[:, half * kspan * 512:(half + 1) * kspan * 512].rearrange(
                        "p (k c) -> p k c", k=kspan
                    ),
                    wqkv.rearrange("(k p) c -> p k c", p=128)[
                        :, half * kspan:(half + 1) * kspan,
                        ch * 512:(ch + 1) * 512
                    ],
                )
            wqkv_ch.append(t)
        wproj_sb = const.tile([128, KT * C], BF16)     # 16KB/part
        nc.sync.dma_start(
            wproj_sb[:].rearrange("p (k c) -> p k c", k=KT),
            wproj.rearrange("(k p) c -> p k c", p=128),
        )
        bias128 = const.tile([128, C], BF16)
        nc.sync.dma_start(bias128[:], bias.broadcast_to([128, C]))

        for w in range(nwin):
            # ---- load xT window: KT tiles [128c, 512 tok] (group-major) ----
            xw = xw0 if w == 0 else load_xw(w)

            # ---- qkv matmuls, group-major output ----
            qb = qpool.tile([128, 4 * C], BF16)   # [g, (n, h, dh)]
            kb = kpool.tile([128, 4 * C], BF16)   # [g, (m, h, dh)]
            vb = vpool.tile([128, 4 * C], BF16)   # [g, (m, h, dh)]
            dest_of = {0: qb, 1: kb, 2: vb}
            # chunk order: all q (attention needs full Q), then k per key
            # position m, then v per m -- so score chain m can start well
            # before the window's qkv finishes.
            def qkv_chunks(order):
                for ch, n in order:
                    ps = ps_qkv.tile([128, 512], F32)
                    for k in range(KT):
                        nc.tensor.matmul(
                            ps[:],
                            lhsT=xw[:, k * WIN + n * 128:(k * WIN) + (n + 1) * 128],
                            rhs=wqkv_ch[ch][:, k * 512:(k + 1) * 512],
                            start=(k == 0),
                            stop=(k == KT - 1),
                        )
                    which, hblk = divmod(ch, 2)
                    dst = dest_of[which][:, n * C + hblk * 512: n * C + (hblk + 1) * 512]
                    nc.scalar.copy(dst, ps[:])

            qkv_chunks([(ch, n) for ch in (0, 1) for n in range(GSZ)])
            qkv_chunks([(ch, n) for n in (2, 0, 1, 3) for ch in (2, 3)])

            # ---- attention (per window, all 16 heads) ----
            # scores: S[g, (m, n, h)] = sum_dh Q[g,n,h,:] * K[g,m,h,:]
            # m chains 0,1 on DVE; 2,3 on GpSimd
            s_f = spool.tile([128, 256], F32, tag="s")
            e_f = spool.tile([128, 256], F32, tag="e")
            # exp is emitted per-m right after each chain (softmax over m,
            # no max-subtraction: |S| is O(5) here); m1's add-tree runs on
            # GpSimd, which idles while DVE is the serial path
            _score_chain(nc.vector, 0, dscr, qb, kb, s_f, exp_out=e_f, nc=nc)
            _score_chain(nc.vector, 1, dscr, qb, kb, s_f, tree_eng=nc.gpsimd,
                         exp_out=e_f, nc=nc)
            _score_chain(nc.gpsimd, 2, pscr, qb, kb, s_f, exp_out=e_f, nc=nc)
            _score_chain(nc.vector, 3, dscr, qb, kb, s_f, exp_out=e_f, nc=nc)

            e_nhm = e_f[:].rearrange("p (m n h) -> p n h m", m=GSZ, n=GSZ)
            z_f = spool.tile([128, 64], F32, tag="z")
            nc.vector.tensor_reduce(
                z_f[:].rearrange("p (n h) -> p n h", n=GSZ), e_nhm,
                axis=AX.X, op=ALU.add,
            )
            r_f = spool.tile([128, 64], F32, tag="r")
            nc.vector.reciprocal(r_f[:], z_f[:])
            pb = spool.tile([128, 256], BF16, tag="pb")  # [g, (n, h, m)]
            r_v = (
                r_f[:].rearrange("p (n h) -> p n h", n=GSZ)
                .unsqueeze(3)
                .broadcast_to([128, GSZ, H, GSZ])
            )
            pb_v = pb[:].rearrange("p (n h m) -> p n h m", n=GSZ, h=H)
            nc.vector.tensor_mul(pb_v, e_nhm, r_v)

            # v chunks emitted after the softmax ops so exp doesn't queue
            # behind the v copies on ACT
            qkv_chunks([(ch, n) for n in range(GSZ) for ch in (4, 5)])

            # AV: O[g, (n, h, d)] = sum_m P[g,n,h,m] * V[g,m,h,:]
            # Split by head-half: heads 0..7 on DVE, 8..15 on GpSimd.
            # Each engine computes its half of all 4 m-products and the
            # add tree, writing its half of ob directly (no combine op).
            ob = opool.tile([128, 4 * C], BF16)
            HH = H // 2

            def pv_mul_half(eng, m, dst_v, h0):
                v_v = (
                    vb[:, m * C + h0 * DH: m * C + (h0 + HH) * DH]
                    .rearrange("p (h d) -> p h d", h=HH)
                    .unsqueeze(1)
                    .broadcast_to([128, GSZ, HH, DH])
                )
                p_v = (
                    pb[:].rearrange("p (n h m) -> p n h m", n=GSZ, h=H)[
                        :, :, h0:h0 + HH, m
                    ]
                    .unsqueeze(3)
                    .broadcast_to([128, GSZ, HH, DH])
                )
                eng.tensor_mul(dst_v, v_v, p_v)

            def av_half(eng, scratch, h0):
                half = lambda t, i: t[:, i * 2048:(i + 1) * 2048].rearrange(
                    "p (n h d) -> p n h d", n=GSZ, h=HH
                )
                t1 = scratch.tile([128, 4 * C], BF16, tag="scrA", bufs=2)
                t2 = scratch.tile([128, 4 * C], BF16, tag="scrA", bufs=2)
                u = scratch.tile([128, 2048], BF16, tag="scrB", bufs=1)
                pv_mul_half(eng, 0, half(t1, 0), h0)
                pv_mul_half(eng, 1, half(t1, 1), h0)
                pv_mul_half(eng, 2, half(t2, 0), h0)
                pv_mul_half(eng, 3, half(t2, 1), h0)
                u_v = u[:].rearrange("p (n h d) -> p n h d", n=GSZ, h=HH)
                eng.tensor_add(u_v, half(t1, 0), half(t1, 1))
                eng.tensor_add(half(t1, 0), half(t2, 0), half(t2, 1))
                ob_v = ob[:].rearrange("p (n h d) -> p n h d", n=GSZ, h=H)[
                    :, :, h0:h0 + HH, :
                ]
                eng.tensor_add(ob_v, u_v, half(t1, 0))

            av_half(nc.vector, dscr, 0)
            av_half(nc.gpsimd, pscr, HH)

            # ---- transpose O to feature-major oT: KT tiles [128c, (n, g)]
            # via the DMA xbar (SBUF->SBUF), keeping the PE free for matmuls ----
            ot = []
            for j in range(KT):
                otj = otpool.tile([128, WIN], BF16, tag=f"ot{j}")
                for n in range(GSZ):
                    nc.sync.dma_start(
                        otj[:, n * 128:(n + 1) * 128],
                        ob[:, n * C + j * 128: n * C + (j + 1) * 128],
                        transpose=True,
                    )
                ot.append(otj)

            # ---- proj matmul; bias added in the PSUM->SBUF copy; DMA out ----
            for n in range(GSZ):
                for ch in range(2):
                    psy = ps_y.tile([128, 512], F32)
                    for k in range(KT):
                        nc.tensor.matmul(
                            psy[:],
                            lhsT=ot[k][:, n * 128:(n + 1) * 128],
                            rhs=wproj_sb[:, k * C + ch * 512: k * C + (ch + 1) * 512],
                            start=(k == 0),
                            stop=(k == KT - 1),
                        )
                    y_t = ypool.tile([128, 512], F32)
                    nc.vector.tensor_add(
                        y_t[:], psy[:], bias128[:, ch * 512:(ch + 1) * 512]
                    )
                    nc.sync.dma_start(
                        y[w * WIN + n: w * WIN + WIN: GSZ,
                          ch * 512:(ch + 1) * 512],
                        y_t[:],
                    )


def build_nc(t_core=T_CORE):
    nc = bacc.Bacc("TRN2", target_bir_lowering=False, debug=False)
    xt_d = nc.dram_tensor("xt", [C, t_core], BF16, kind="ExternalInput")
    wqkv_d = nc.dram_tensor("wqkv", [C, OUT3], BF16, kind="ExternalInput")
    wproj_d = nc.dram_tensor("wproj", [C, C], BF16, kind="ExternalInput")
    bias_d = nc.dram_tensor("bias", [1, C], BF16, kind="ExternalInput")
    y_d = nc.dram_tensor("y", [t_core, C], F32, kind="ExternalOutput")
    with tile.TileContext(nc) as tc:
        group_attn_kernel(
            tc, y_d[:], xt_d[:], wqkv_d[:], wproj_d[:], bias_d[:],
            t_core=t_core,
        )
    nc.compile()
    return nc


def make_in_maps(x, w_qkv, w_proj, b_proj):
    bf = ml_dtypes.bfloat16
    xf = np.ascontiguousarray(np.asarray(x, dtype=np.float32)).reshape(-1, C)
    wq = np.array(w_qkv, dtype=np.float32, copy=True)
    wq[:, :C] *= DH ** -0.5  # fold attention scale into q columns
    wqb = wq.astype(bf)
    wpb = np.asarray(w_proj, dtype=np.float32).astype(bf)
    bb = np.asarray(b_proj, dtype=np.float32).reshape(1, C).astype(bf)
    xb = xf.astype(bf)
    in_maps = []
    for i in range(NCORES):
        xc = xb[i * T_CORE:(i + 1) * T_CORE]          # [4096, C]
        # feature-major with group-major token order: col = w*512+n*128+gi
        # for token w*512+gi*4+n
        xg = xc.reshape(T_CORE // WIN, G128, GSZ, C)  # [w, gi, n, c]
        xg = np.ascontiguousarray(np.transpose(xg, (3, 0, 2, 1)))
        in_maps.append({
            "xt": xg.reshape(C, T_CORE),
            "wqkv": wqb,
            "wproj": wpb,
            "bias": bb,
        })
    return in_maps


_NC_CACHE = {}


def _get_nc():
    if "nc" not in _NC_CACHE:
        _NC_CACHE["nc"] = build_nc()
    return _NC_CACHE["nc"]


def kernel(x, w_qkv, w_proj, b_proj, causal=0, **_unused):
    nc = _get_nc()
    in_maps = make_in_maps(x, w_qkv, w_proj, b_proj)
    res = run_bass_kernel_spmd(nc, in_maps, core_ids=list(range(NCORES)))
    y = np.concatenate([r["y"] for r in res.results], axis=0)
    return y.reshape(B, N, C).astype(np.float32)
